# revision 3
# baseline (speedup 1.0000x reference)
"""GroupedQueryAttention (B=1, S=2048, D=4096, 32 Q heads / 8 KV heads) on 8 TRN2 cores.

Sharding: one KV group (4 Q heads + 1 KV head) per core.  Per core:
  - QKV projection for its head group (q^T/k^T/v^T orientation: dims on partitions)
  - RoPE on q (scale folded in) and k via DVE
  - causal flash-style attention in scores^T orientation:
      scoresT[t,s] tiles from PE, mask-add (diag), exp on ACT -> P^T (bf16),
      denominator = DVE partial-sum accum + ones-matmul partition reduce,
      AV accumulation out^T[d,s] on PE, normalize via reciprocal+ones-broadcast.
  - AllToAll swaps head-dims for seq-slices: each core ends with the FULL
    attention output (all 4096 dims) for its 256-row seq slice.
  - out-projection against full Wo -> out[256, 4096]; host concatenates + bias.

All matmuls bf16 inputs / fp32 PSUM accumulation (measured rel_l2 ~7e-3 vs f64 ref).
"""
import numpy as np
import ml_dtypes

from concourse import bass, bacc, tile, mybir
from concourse.bass_utils import run_bass_kernel_spmd

BF16 = ml_dtypes.bfloat16
F32 = np.float32

D = 4096          # model dim
S = 2048          # sequence
NH = 32           # query heads
NG = 8            # kv heads == n cores
HD = 128          # head dim
G = NH // NG      # 4 query heads per group/core
KV = NG * HD      # 1024
BASE = 50000.0
SCALE = 1.0 / np.sqrt(HD)
N_CORES = 8
SC = S // 512     # 4 s-chunks of 512
MC = D // 128     # 32 contraction chunks
SSLICE = S // N_CORES  # 256 rows of final output per core

_CACHE = {}


def _build(reps: int = 1, sim: bool = False):
    f32 = mybir.dt.float32
    f32r = mybir.dt.float32r
    bf16 = mybir.dt.bfloat16

    nc = bacc.Bacc("TRN2", target_bir_lowering=False, debug=False,
                   num_devices=N_CORES)

    # ---- I/O ----
    xt_d = nc.dram_tensor("xt", [128, SC * MC * 512], bf16, kind="ExternalInput")
    wq_d = nc.dram_tensor("wq", [128, MC * 512], bf16, kind="ExternalInput")
    wk_d = nc.dram_tensor("wk", [128, MC * 128], bf16, kind="ExternalInput")
    wv_d = nc.dram_tensor("wv", [128, MC * 128], bf16, kind="ExternalInput")
    wo_d = nc.dram_tensor("wo", [128, 8 * MC * 512], bf16, kind="ExternalInput")
    cosq_d = nc.dram_tensor("cosq", [128, S], bf16, kind="ExternalInput")
    sinq_d = nc.dram_tensor("sinq", [128, S], bf16, kind="ExternalInput")
    cosk_d = nc.dram_tensor("cosk", [128, S], bf16, kind="ExternalInput")
    sink_d = nc.dram_tensor("sink", [128, S], bf16, kind="ExternalInput")
    bq_d = nc.dram_tensor("bq", [128, G], f32, kind="ExternalInput")
    bk_d = nc.dram_tensor("bk", [128, 1], f32, kind="ExternalInput")
    bv_d = nc.dram_tensor("bv", [128, 1], f32, kind="ExternalInput")
    mask_d = nc.dram_tensor("mask", [128, 4 * 512], f32, kind="ExternalInput")
    ident_d = nc.dram_tensor("ident", [128, 128], bf16, kind="ExternalInput")
    onem_d = nc.dram_tensor("onem", [128, 128], bf16, kind="ExternalInput")
    out_d = nc.dram_tensor("out", [SSLICE, D], f32, kind="ExternalOutput")

    Ident = mybir.ActivationFunctionType.Identity
    CopyF = mybir.ActivationFunctionType.Copy
    Exp = mybir.ActivationFunctionType.Exp
    mult = mybir.AluOpType.mult

    with tile.TileContext(nc) as tc:
        with tc.tile_pool(name="const", bufs=1) as cp, \
             tc.tile_pool(name="pers", bufs=1) as pp, \
             tc.tile_pool(name="dram", bufs=1, space="DRAM") as dramp:
            # constants
            bq = cp.tile([128, G], f32); nc.sync.dma_start(bq[:], bq_d[:])
            bk = cp.tile([128, 1], f32); nc.sync.dma_start(bk[:], bk_d[:])
            bv = cp.tile([128, 1], f32); nc.sync.dma_start(bv[:], bv_d[:])
            mask = cp.tile([128, 4 * 512], f32); nc.sync.dma_start(mask[:], mask_d[:])
            ident = cp.tile([128, 128], bf16); nc.sync.dma_start(ident[:], ident_d[:])
            onem = cp.tile([128, 128], bf16); nc.sync.dma_start(onem[:], onem_d[:])

            # persistent per-rep intermediates
            qT = pp.tile([128, G * S], bf16)      # rope'd q^T, head h at [:, h*S:]
            kT = pp.tile([128, S], bf16)
            vN = pp.tile([128, S], bf16)          # v natural, t-chunk tt at [:, tt*128:]

            for _rep in range(reps):
                # ================= QKV projection =================
                with tc.tile_pool(name="wqkv", bufs=1) as wp, \
                     tc.tile_pool(name="xs", bufs=2) as xsp, \
                     tc.tile_pool(name="rtmp", bufs=2) as rtp, \
                     tc.tile_pool(name="rope_ps", bufs=2, space="PSUM") as rpp, \
                     tc.tile_pool(name="qkv_ps", bufs=4, space="PSUM") as qps:
                    wq = wp.tile([128, MC * 512], bf16)
                    nc.sync.dma_start(wq[:], wq_d[:])
                    wk = wp.tile([128, MC * 128], bf16)
                    nc.sync.dma_start(wk[:], wk_d[:])
                    wv = wp.tile([128, MC * 128], bf16)
                    nc.sync.dma_start(wv[:], wv_d[:])
                    cosq = wp.tile([128, S], bf16); nc.sync.dma_start(cosq[:], cosq_d[:])
                    sinq = wp.tile([128, S], bf16); nc.sync.dma_start(sinq[:], sinq_d[:])
                    cosk = wp.tile([128, S], bf16); nc.sync.dma_start(cosk[:], cosk_d[:])
                    sink = wp.tile([128, S], bf16); nc.sync.dma_start(sink[:], sink_d[:])
                    vTt = wp.tile([128, S], bf16)   # v^T (pre-transpose)

                    def rope(dst, ps, bias_ap, cos_t, sin_t, sc):
                        # dst = b+ps)*cos + swap64(b+ps)*sin_signed  (all [128,512])
                        # partition-crossing reads must come from PSUM: walrus
                        # rejects SB+SB tensor_tensor with mismatched base partition
                        cs = slice(sc * 512, (sc + 1) * 512)
                        ps2 = rpp.tile([128, 512], f32, name="rope_ps2")
                        nc.scalar.activation(ps2[:], ps[:], Ident, bias=bias_ap)
                        t1 = rtp.tile([128, 512], f32, name="rope_t1")
                        nc.vector.tensor_tensor(t1[:], ps2[:], cos_t[:, cs], mult)
                        t2 = rtp.tile([128, 512], f32, name="rope_t2")
                        nc.vector.tensor_tensor(t2[0:64, :], ps2[64:128, :],
                                                sin_t[0:64, cs], mult)
                        nc.vector.tensor_tensor(t2[64:128, :], ps2[0:64, :],
                                                sin_t[64:128, cs], mult)
                        nc.vector.tensor_add(dst, t1[:], t2[:])

                    for sc in range(SC):
                        xs = xsp.tile([128, MC * 512], bf16, name="xs")
                        nc.sync.dma_start(
                            xs[:], xt_d[:, sc * MC * 512:(sc + 1) * MC * 512])
                        for blk in range(G + 2):  # 4 q heads, then k, then v
                            ps = qps.tile([128, 512], f32, name="qkv_ps")
                            for mc in range(MC):
                                if blk < G:
                                    lhsT = wq[:, mc * 512 + 128 * blk:
                                              mc * 512 + 128 * blk + 128]
                                elif blk == G:
                                    lhsT = wk[:, mc * 128:(mc + 1) * 128]
                                else:
                                    lhsT = wv[:, mc * 128:(mc + 1) * 128]
                                nc.tensor.matmul(
                                    ps[:], lhsT,
                                    xs[:, mc * 512:(mc + 1) * 512],
                                    start=(mc == 0), stop=(mc == MC - 1))
                            cs = slice(sc * 512, (sc + 1) * 512)
                            if blk < G:
                                rope(qT[:, blk * S + sc * 512:
                                         blk * S + sc * 512 + 512],
                                     ps, bq[:, blk:blk + 1], cosq, sinq, sc)
                            elif blk == G:
                                rope(kT[:, cs], ps, bk[:, 0:1], cosk, sink, sc)
                            else:
                                nc.scalar.activation(vTt[:, cs], ps[:], Ident,
                                                     bias=bv[:, 0:1])
                    # transpose v^T -> v natural
                    for tt in range(S // 128):
                        pst = qps.tile([128, 128], bf16, name="tr_ps", bufs=2)
                        nc.tensor.transpose(pst[:], vTt[:, tt * 128:(tt + 1) * 128],
                                            ident[:])
                        nc.vector.tensor_copy(vN[:, tt * 128:(tt + 1) * 128], pst[:])

                # ================= attention =================
                # wo streaming pool opened here so its DMAs can prefetch during attention
                with tc.tile_pool(name="wo_sb", bufs=2) as wop:
                    attnT = wop.tile([128, G * S], bf16, bufs=1)  # attn out^T
                    with tc.tile_pool(name="es_sb", bufs=4) as esp, \
                         tc.tile_pool(name="acc_sb", bufs=2) as accp, \
                         tc.tile_pool(name="qk_ps", bufs=2, space="PSUM") as qkp, \
                         tc.tile_pool(name="av_ps", bufs=2, space="PSUM") as avp, \
                         tc.tile_pool(name="b_ps", bufs=2, space="PSUM") as bpp:
                        for c in range(SC):
                            ss = slice(c * 512, (c + 1) * 512)
                            for h in range(G):
                                av = avp.tile([128, 512], f32, name="av")
                                acc = accp.tile([128, 512], f32, name="acc")
                                ntt = 4 * (c + 1)
                                for tt in range(ntt):
                                    qk = qkp.tile([128, 512], f32, name="qk")
                                    nc.tensor.matmul(
                                        qk[:], kT[:, tt * 128:(tt + 1) * 128],
                                        qT[:, h * S + c * 512:h * S + c * 512 + 512],
                                        start=True, stop=True)
                                    if tt >= 4 * c:
                                        r = tt - 4 * c
                                        nc.vector.tensor_add(
                                            qk[:], qk[:],
                                            mask[:, r * 512:(r + 1) * 512])
                                    es = esp.tile([128, 512], bf16, name="es")
                                    nc.scalar.activation(es[:], qk[:], Exp)
                                    if tt == 0:
                                        nc.vector.tensor_copy(acc[:], es[:])
                                    else:
                                        nc.vector.tensor_add(acc[:], acc[:], es[:])
                                    nc.tensor.matmul(
                                        av[:], vN[:, tt * 128:(tt + 1) * 128], es[:],
                                        start=(tt == 0), stop=(tt == ntt - 1))
                                # denom: all-ones [128,128] matmul = partition
                                # reduce + broadcast in one standard-shape op
                                accb = esp.tile([128, 512], bf16, name="accb",
                                                bufs=2)
                                nc.vector.tensor_copy(accb[:], acc[:])
                                bps = bpp.tile([128, 512], f32, name="bps")
                                nc.tensor.matmul(bps[:], onem[:], accb[:],
                                                 start=True, stop=True)
                                recb = esp.tile([128, 512], f32, name="recb",
                                                bufs=2)
                                nc.vector.reciprocal(recb[:], bps[:])
                                nc.vector.tensor_tensor(
                                    attnT[:, h * S + c * 512:h * S + c * 512 + 512],
                                    av[:], recb[:], mult)

                    # ================= AllToAll =================
                    bnc_in = dramp.tile([D, SSLICE], bf16, name="bnc_in")
                    bnc_out = dramp.tile([D, SSLICE], bf16, name="bnc_out")
                    for j in range(N_CORES):
                        for h in range(G):
                            nc.sync.dma_start(
                                bnc_in[512 * j + 128 * h:512 * j + 128 * h + 128, :],
                                attnT[:, h * S + SSLICE * j:
                                      h * S + SSLICE * j + SSLICE])
                    if sim:
                        # timeline-sim stand-in for the A2A: same-size DRAM copy
                        nc.sync.dma_start(bnc_out[:], bnc_in[:])
                    else:
                        nc.gpsimd.collective_compute(
                            "AllToAll", mybir.AluOpType.bypass,
                            replica_groups=[list(range(N_CORES))],
                            ins=[bnc_in.opt()], outs=[bnc_out.opt()])

                    # ================= out projection =================
                    with tc.tile_pool(name="wo_phase", bufs=1) as wph, \
                         tc.tile_pool(name="o_ps", bufs=3, space="PSUM") as ops:
                        attnF = wph.tile([128, MC * SSLICE], bf16)
                        for mc in range(MC):
                            nc.sync.dma_start(
                                attnF[:, mc * SSLICE:(mc + 1) * SSLICE],
                                bnc_out[mc * 128:(mc + 1) * 128, :])
                        out_sb = []
                        for s2 in range(2):
                            t = wph.tile([128, D], f32, name=f"out_sb{s2}")
                            out_sb.append(t)
                        for e in range(8):
                            wo = wop.tile([128, MC * 512], bf16, name="wo_t")
                            nc.sync.dma_start(
                                wo[:], wo_d[:, e * MC * 512:(e + 1) * MC * 512])
                            for s2 in range(2):
                                po = ops.tile([128, 512], f32, name="po")
                                for mc in range(MC):
                                    nc.tensor.matmul(
                                        po[:],
                                        attnF[:, mc * SSLICE + 128 * s2:
                                              mc * SSLICE + 128 * s2 + 128],
                                        wo[:, mc * 512:(mc + 1) * 512],
                                        start=(mc == 0), stop=(mc == MC - 1))
                                nc.scalar.activation(
                                    out_sb[s2][:, e * 512:(e + 1) * 512],
                                    po[:], CopyF)
                        for s2 in range(2):
                            nc.sync.dma_start(out_d[128 * s2:128 * (s2 + 1), :],
                                              out_sb[s2][:])
    nc.compile()
    return nc


def _prep_inputs(x, Wqkv, bqkv, Wo, bo):
    """Host-side shard prep. Returns in_maps for the 8 cores."""
    x0T = np.ascontiguousarray(np.asarray(x, F32)[0].T)          # [D, S]
    # [p, sc, mc, c] tiling of x^T
    xt_t = np.ascontiguousarray(
        x0T.reshape(MC, 128, SC, 512).transpose(1, 2, 0, 3)
    ).reshape(128, SC * MC * 512).astype(BF16)

    Wqkv = np.asarray(Wqkv, F32)
    Wo = np.asarray(Wo, F32)
    bqkv = np.asarray(bqkv, F32)

    wo_t = np.ascontiguousarray(
        Wo.reshape(MC, 128, 8, 512).transpose(1, 2, 0, 3)
    ).reshape(128, 8 * MC * 512).astype(BF16)

    # rope tables (transposed: [HD, S]); q tables fold in 1/sqrt(HD)
    inv_freq = 1.0 / (BASE ** (np.arange(0, HD, 2, dtype=np.float64) / HD))
    t = np.arange(S, dtype=np.float64)
    freqs = np.outer(t, inv_freq)                                # [S, 64]
    emb = np.concatenate([freqs, freqs], axis=1)                 # [S, HD]
    cosT = np.cos(emb).T.astype(F32)                             # [HD, S]
    sinT = np.sin(emb).T.astype(F32)
    sin_signed = np.concatenate([-sinT[:64], sinT[64:]], axis=0)
    cosq = np.ascontiguousarray(cosT * SCALE).astype(BF16)
    sinq = np.ascontiguousarray(sin_signed * SCALE).astype(BF16)
    cosk = np.ascontiguousarray(cosT).astype(BF16)
    sink = np.ascontiguousarray(sin_signed).astype(BF16)

    # causal mask patterns for the 4 diagonal t-tiles of each 512-wide s-chunk
    p = np.arange(128)[:, None]
    f = np.arange(512)[None, :]
    mask = np.stack([np.where(128 * r + p <= f, 0.0, -1e30) for r in range(4)],
                    axis=1).astype(F32).reshape(128, 4 * 512)

    ident = np.eye(128, dtype=np.float32).astype(BF16)
    onem = np.ones((128, 128), BF16)

    in_maps = []
    for g in range(N_CORES):
        wq_g = np.ascontiguousarray(
            Wqkv[:, 512 * g:512 * (g + 1)].reshape(MC, 128, 512)
            .transpose(1, 0, 2)).reshape(128, MC * 512).astype(BF16)
        wk_g = np.ascontiguousarray(
            Wqkv[:, D + 128 * g:D + 128 * (g + 1)].reshape(MC, 128, 128)
            .transpose(1, 0, 2)).reshape(128, MC * 128).astype(BF16)
        wv_g = np.ascontiguousarray(
            Wqkv[:, D + KV + 128 * g:D + KV + 128 * (g + 1)]
            .reshape(MC, 128, 128).transpose(1, 0, 2)
        ).reshape(128, MC * 128).astype(BF16)
        bq_g = np.ascontiguousarray(
            bqkv[512 * g:512 * (g + 1)].reshape(G, 128).T).astype(F32)
        bk_g = bqkv[D + 128 * g:D + 128 * (g + 1)].reshape(128, 1).astype(F32)
        bv_g = bqkv[D + KV + 128 * g:D + KV + 128 * (g + 1)].reshape(128, 1).astype(F32)
        in_maps.append({
            "xt": xt_t, "wq": wq_g, "wk": wk_g, "wv": wv_g, "wo": wo_t,
            "cosq": cosq, "sinq": sinq, "cosk": cosk, "sink": sink,
            "bq": bq_g, "bk": bk_g, "bv": bv_g, "mask": mask,
            "ident": ident, "onem": onem,
        })
    return in_maps


def kernel(x, Wqkv, bqkv, Wo, bo):
    if "nc" not in _CACHE:
        _CACHE["nc"] = _build(reps=1)
    nc = _CACHE["nc"]
    in_maps = _prep_inputs(x, Wqkv, bqkv, Wo, bo)
    res = run_bass_kernel_spmd(nc, in_maps, core_ids=list(range(N_CORES)))
    out = np.concatenate([res.results[g]["out"] for g in range(N_CORES)], axis=0)
    out = out + np.asarray(bo, F32)[None, :]
    return out[None].astype(F32)

